# revision 40
# baseline (speedup 1.0000x reference)
"""DiffCLIP differential-attention block on 8 Trainium2 NeuronCores, v3.

Sharding: the (batch=4) x (head-group=2) grid maps to the 8 cores — core
c = 2*b + g handles batch b and half the heads (4 of 8 effective heads),
i.e. a 512-column slice of the q/k/v projections and the matching 512-row
slice of the out projection. Each core emits a partial (L, D) output; the
host sums the two per-batch partials and stacks.

v3 changes vs v2:
  - K=64 score matmuls emitted as adjacent row-group pairs
    (tile_position (0,0)/(64,0)) so the PE runs both concurrently
  - RMS tail is DRAM-round-trip-free: rsqrt computed as exp(-0.5*ln(w))
    on partition-replicated [128, L] tiles — keeps ScalarE on the single
    natural_log_exp_and_others table set (no ~2.7us table reloads)
  - fine-grained software pipeline: qkT/v/u matmul "units" interleaved
    into the score/exp stream so the PE never idles long enough for the
    HAM clock gate to re-throttle
  - out-projection starts its g=0..2 PSUM accumulation while the last
    pair's tail still computes; y evacuations split across ACT and DVE
  - weight DMAs ordered so the first matmul's operands arrive first
"""

import sys

if "/opt/trn_rl_repo" not in sys.path:
    sys.path.insert(0, "/opt/trn_rl_repo")

import numpy as np
import ml_dtypes

L, D, H, HD, HE = 1024, 1024, 16, 64, 8
LAMBDA_INIT = 0.8
EPS = 1e-5
NB = 4
NCORES = 8
COLS = 512  # per-core projection column count

LAST_RESULT = None  # BassKernelResults of the most recent kernel() call


def _split_excess_waits(nc, max_waits: int = 1):
    """Walrus codegen on this toolchain accepts at most one sync-wait command
    per hardware instruction (plus its update); Tile freely emits several.
    Split the excess waits onto preceding same-engine NoOps."""
    import bass_rust
    import concourse.mybir as mybir

    for f in nc.m.functions:
        for blk in f.blocks:
            insts = blk.instructions
            out = []
            changed = False
            for inst in insts:
                si = inst.sync_info
                if si is not None and si.on_wait and len(si.on_wait) > max_waits:
                    waits = list(si.on_wait)
                    for j, w in enumerate(waits[max_waits:]):
                        nop = mybir.InstNoOp(
                            name=f"{inst.name}-xw{j}",
                            sync_info=bass_rust.SyncInfo(
                                on_wait=[w], on_update=[]
                            ),
                            bass_nofuse=True,
                            engine=inst.engine,
                        )
                        nc.register_instruction(nop, overwrite=True)
                        out.append(nop)
                    inst.sync_info = bass_rust.SyncInfo(
                        on_wait=waits[:max_waits],
                        on_update=list(si.on_update or []),
                    )
                    changed = True
                out.append(inst)
            if changed:
                blk.instructions = out


def _build(lam: float, with_mask: bool, with_qk_bias: bool, with_v_bias: bool,
           split_waits: bool = True):
    import concourse.bass as bass
    import concourse.tile as tile
    import concourse.mybir as mybir

    bf16 = mybir.dt.bfloat16
    f32 = mybir.dt.float32
    AF = mybir.ActivationFunctionType
    ALU = mybir.AluOpType

    SQRT_EPS = float(np.sqrt(EPS))

    nc = bass.Bass()
    xT_d = nc.dram_tensor("xT", [D, L], bf16, kind="ExternalInput")
    wq_d = nc.dram_tensor("wq", [D, COLS], bf16, kind="ExternalInput")
    wk_d = nc.dram_tensor("wk", [D, COLS], bf16, kind="ExternalInput")
    wv_d = nc.dram_tensor("wv", [D, COLS], bf16, kind="ExternalInput")
    wo_d = nc.dram_tensor("wo", [COLS, D], bf16, kind="ExternalInput")
    if with_qk_bias:
        bq_d = nc.dram_tensor("bqs", [COLS], f32, kind="ExternalInput")
        bk_d = nc.dram_tensor("bks", [COLS], f32, kind="ExternalInput")
    if with_v_bias:
        bv_d = nc.dram_tensor("bvs", [COLS], f32, kind="ExternalInput")
    if with_mask:
        maskT_d = nc.dram_tensor("maskT", [L, L], bf16, kind="ExternalInput")
    y_d = nc.dram_tensor("y", [L, D], f32, kind="ExternalOutput")

    with tile.TileContext(nc) as tc:
        persist = tc.alloc_tile_pool(name="persist", bufs=1)
        qT = persist.tile([128, 4, L], bf16)
        kT = persist.tile([128, 4, L], bf16)
        v = persist.tile([128, 8, COLS], bf16)
        wo_s = persist.tile([128, 4, D], bf16)
        outT = persist.tile([128, 4, L], bf16)
        ones = persist.tile([128, 128], bf16)
        nc.vector.memset(ones, 1.0)
        if with_mask:
            ident = persist.tile([128, 128], bf16)
            from concourse.masks import make_identity
            make_identity(nc, ident)
            maskT_s = persist.tile([128, 8, L], bf16)
            nc.sync.dma_start(
                maskT_s, maskT_d.rearrange("(ko p) l -> p ko l", p=128)
            )

        with (
            tc.tile_pool(name="stage_a", bufs=1) as sa,
            tc.tile_pool(name="epool", bufs=2) as ep,
            tc.tile_pool(name="sums", bufs=2) as sp,
            tc.tile_pool(name="t4p", bufs=1) as t4p,
            tc.tile_pool(name="tailp", bufs=2) as tp,
            tc.tile_pool(name="psA", bufs=2, space="PSUM") as psA,
            tc.tile_pool(name="psS", bufs=2, space="PSUM") as psS,
            tc.tile_pool(name="psU", bufs=2, space="PSUM") as psU,
        ):
            xts = sa.tile([128, 8, L], bf16)
            wq_s = sa.tile([128, 8, COLS], bf16)
            wk_s = sa.tile([128, 8, COLS], bf16)
            wv_s = sa.tile([128, 8, COLS], bf16)
            xT_r = xT_d.rearrange("(ko p) l -> p ko l", p=128)
            wq_r = wq_d.rearrange("(ko p) m -> p ko m", p=128)
            wk_r = wk_d.rearrange("(ko p) m -> p ko m", p=128)
            wv_r = wv_d.rearrange("(ko p) m -> p ko m", p=128)
            # x split across sync+gpsimd queues; g0 slices of wq/wk first on
            # scalar/vector queues so the first matmuls' operands land early.
            # x spread over all three DMA-capable queues, g0 weight slices
            # first so the opening qkT units are never transfer-starved
            nc.scalar.dma_start(wq_s[:, :, 0:128], wq_r[:, :, 0:128])
            for kb in range(3):
                nc.sync.dma_start(xts[:, kb], xT_r[:, kb])
            for kb in range(3, 6):
                nc.gpsimd.dma_start(xts[:, kb], xT_r[:, kb])
            nc.scalar.dma_start(wk_s[:, :, 0:128], wk_r[:, :, 0:128])
            nc.scalar.dma_start(xts[:, 6], xT_r[:, 6])
            nc.scalar.dma_start(xts[:, 7], xT_r[:, 7])
            nc.sync.dma_start(wq_s[:, :, 128:512], wq_r[:, :, 128:512])
            nc.gpsimd.dma_start(wk_s[:, :, 128:512], wk_r[:, :, 128:512])
            nc.gpsimd.dma_start(wv_s[:], wv_r[:])
            nc.sync.dma_start(wo_s, wo_d.rearrange("(ko p) n -> p ko n", p=128))
            if with_qk_bias:
                bq_s = sa.tile([128, 4], f32)
                bk_s = sa.tile([128, 4], f32)
                nc.sync.dma_start(bq_s, bq_d.rearrange("(mb p) -> p mb", p=128))
                nc.sync.dma_start(bk_s, bk_d.rearrange("(mb p) -> p mb", p=128))
            if with_v_bias:
                bv_s = sa.tile([128, COLS], f32)
                bv_ap = bv_d[:]
                nc.gpsimd.dma_start(
                    bv_s,
                    bass.AP(
                        tensor=bv_ap.tensor,
                        offset=bv_ap.offset,
                        ap=[[0, 128], list(bv_ap.ap[0])],
                    ),
                )

            # ------------- emission building blocks -------------

            def evac_qk(which, g, lc, acc):
                dst = qT if which == "q" else kT
                dst_ap = dst[:, g, lc * 512:(lc + 1) * 512]
                if with_qk_bias:
                    b_s = bq_s if which == "q" else bk_s
                    nc.scalar.activation(
                        out=dst_ap, in_=acc[:], func=AF.Identity,
                        bias=b_s[:, g:g + 1], scale=1.0,
                    )
                else:
                    nc.vector.tensor_copy(dst_ap, acc[:])

            def unit_qk(g, which, lc):
                # one [128 ch, 512 tok] slice of qT/kT for group g
                wt_s = wq_s if which == "q" else wk_s
                acc = psA.tile([128, 512], f32, tag="acc")
                for kb in range(8):
                    nc.tensor.matmul(
                        acc[:],
                        wt_s[:, kb, g * 128:(g + 1) * 128],
                        xts[:, kb, lc * 512:(lc + 1) * 512],
                        start=(kb == 0),
                        stop=(kb == 7),
                    )
                evac_qk(which, g, lc, acc)

            def emit_qk0_head(g):
                # opening qkT in k-chunk-major order across four parallel
                # accumulators (borrowing the idle psU slots) so each
                # arriving x chunk feeds four matmuls immediately — hides
                # the x transfer tail behind compute
                slots = [("q", 0, psA), ("k", 0, psA), ("q", 1, psU),
                         ("k", 1, psU)]
                accs = []
                for which, lc, pool in slots:
                    accs.append(pool.tile(
                        [128, 512], f32,
                        tag="acc" if pool is psA else "u",
                        name=f"qk0_{which}_{lc}"))
                for kb in range(8):
                    for (which, lc, pool), acc in zip(slots, accs):
                        wt_s = wq_s if which == "q" else wk_s
                        nc.tensor.matmul(
                            acc[:],
                            wt_s[:, kb, g * 128:(g + 1) * 128],
                            xts[:, kb, lc * 512:(lc + 1) * 512],
                            start=(kb == 0),
                            stop=(kb == 7),
                        )
                for (which, lc, pool), acc in zip(slots, accs):
                    evac_qk(which, g, lc, acc)

            def unit_v(lb):
                # v rows [128 tok chunk lb, 512 ch]
                acc = psA.tile([128, 512], f32, tag="acc")
                for kb in range(8):
                    nc.tensor.matmul(
                        acc[:],
                        xts[:, kb, lb * 128:(lb + 1) * 128],
                        wv_s[:, kb, :],
                        start=(kb == 0),
                        stop=(kb == 7),
                    )
                if with_v_bias:
                    nc.vector.tensor_add(v[:, lb, :], acc[:], bv_s[:])
                else:
                    nc.vector.tensor_copy(v[:, lb, :], acc[:])

            def emit_score_mm(g, s, kb, lc, sc):
                nc.tensor.matmul(
                    sc[:, lc * 512:(lc + 1) * 512],
                    kT[64 * s:64 * (s + 1), g, kb * 128:(kb + 1) * 128],
                    qT[64 * s:64 * (s + 1), g, lc * 512:(lc + 1) * 512],
                    start=True,
                    stop=not with_mask,
                    tile_position=(64 * s, 0),
                )
                if with_mask:
                    nc.tensor.matmul(
                        sc[:, lc * 512:(lc + 1) * 512],
                        ident[:],
                        maskT_s[:, kb, lc * 512:(lc + 1) * 512],
                        start=False,
                        stop=True,
                    )

            def emit_scores_kb(g, kb, es_full):
                # all four K=64 score matmuls of this k-chunk land in ONE
                # 4-bank PSUM tile: only the quartet's first matmul carries
                # the tile's drain wait, so the following row-group-disjoint
                # partners issue wait-free and the PE packs each pair
                # concurrently (observed: a waiting matmul never co-issues).
                sc = psS.tile([128, 2 * L], f32, tag="sc", bufs=1,
                              name=f"sc_{g}_{kb}")
                for lc in range(2):
                    for s in range(2):
                        emit_score_mm(g, s, kb, lc, sc[:, s * L:(s + 1) * L])
                # ONE exp drains the whole 4-bank tile: the next chunk's
                # quartet then waits on a single older event, and the 2048
                # free-dim amortizes the ACT fixed overhead
                nc.scalar.activation(
                    out=es_full[:, kb, :], in_=sc[:], func=AF.Exp,
                )

            def emit_lvl1(g, es, t4s, s, j):
                # first-level pair add of the row-sum tree (bf16, 2x DVE)
                nc.vector.tensor_add(
                    t4s[s][:, j], es[s][:, 2 * j], es[s][:, 2 * j + 1]
                )

            def emit_sums(g, t4s):
                # finish tree in-place, then ones-matmul partition reduction
                # leaves row sums replicated across partitions. The copies
                # fold in a sqrt(eps) scale: S' = sqrt(eps)*S makes the RMS
                # eps-term simply (S0'*S1')^2 (the S scale itself cancels
                # between z and the denominator), and bf16 S is safe for the
                # same cancellation reason.
                srep_ps = psS.tile([128, 2 * L], f32, tag="sc", bufs=1,
                                   name=f"srep_ps_{g}")
                reps = []
                for s in range(2):
                    t4 = t4s[s]
                    nc.vector.tensor_add(t4[:, 0], t4[:, 0], t4[:, 1])
                    nc.vector.tensor_add(t4[:, 2], t4[:, 2], t4[:, 3])
                    nc.vector.tensor_add(t4[:, 0], t4[:, 0], t4[:, 2])
                    for lc in range(2):
                        nc.tensor.matmul(
                            srep_ps[:, s * L + lc * 512:s * L + (lc + 1) * 512],
                            ones[:],
                            t4[:, 0, lc * 512:(lc + 1) * 512],
                            start=True,
                            stop=True,
                        )
                    srep = sp.tile([128, L], bf16, tag=f"s{s}",
                                   name=f"srep_{g}_{s}")
                    with nc.allow_low_precision(reason="S scale cancels"):
                        nc.scalar.activation(
                            out=srep[:], in_=srep_ps[:, s * L:(s + 1) * L],
                            func=AF.Identity, scale=SQRT_EPS,
                        )
                    reps.append(srep)
                return reps

            def unit_u_half(g, lc, s, es):
                # u_s = v^T e_s for one 512-token q chunk, one s stream
                cs = slice(lc * 512, (lc + 1) * 512)
                pool = psA if (g == 3 and lc == 1) else psU
                u = pool.tile([128, 512], f32,
                              tag="acc" if pool is psA else "u",
                              name=f"u_{g}_{lc}_{s}")
                for kb in range(8):
                    nc.tensor.matmul(
                        u[:],
                        v[:, kb, 128 * g:128 * (g + 1)],
                        es[s][:, kb, cs],
                        start=(kb == 0),
                        stop=(kb == 7),
                    )
                return u

            def emit_combine(g, lc, us, reps, z):
                cs = slice(lc * 512, (lc + 1) * 512)
                s0_rep, s1_rep = reps
                tt = tp.tile([128, 512], bf16, tag="tt", name=f"tt_{g}_{lc}")
                with nc.allow_low_precision(reason="bf16 z, scale cancels"):
                    nc.vector.tensor_mul(z[:, cs], us[0][:], s1_rep[:, cs])
                    nc.vector.scalar_tensor_tensor(
                        out=tt[:], in0=us[1][:], scalar=lam,
                        in1=s0_rep[:, cs], op0=ALU.mult, op1=ALU.mult,
                    )
                    nc.vector.tensor_sub(z[:, cs], z[:, cs], tt[:])

            def emit_tail_ctwt(g, reps):
                # (S0'*S1')^2 = eps*(S0*S1)^2 on replicated tiles (the
                # sqrt(eps) scale was folded into the S copies); depends
                # only on sums, so it can be emitted before the combines.
                s0_rep, s1_rep = reps
                ct = tp.tile([128, L], bf16, tag="ct", name=f"ct_{g}")
                wt = tp.tile([128, L], bf16, tag="wt", name=f"wt_{g}")
                with nc.allow_low_precision(reason="eps term only"):
                    nc.vector.tensor_mul(ct[:], s1_rep[:], s0_rep[:])
                    nc.vector.tensor_mul(wt[:], ct[:], ct[:])
                return ct, wt

            def emit_tail(g, z, reps, ctwt=None):
                # headwise RMS over the 128-partition channel dim, all on
                # replicated [128, L] tiles: out = z * rsqrt(sum(z^2)/128
                # + eps*(S0*S1)^2), with rsqrt = exp(-0.5*ln(.)) so ScalarE
                # stays on the natural_log_exp table set. The z^2 partition
                # sums go through psU tiles so the score-PSUM banks drain
                # early for the output projection's early start.
                if ctwt is None:
                    ctwt = emit_tail_ctwt(g, reps)
                ct, wt = ctwt
                zsq = tp.tile([128, L], bf16, tag="zr", name=f"zsq_{g}")
                lnw = tp.tile([128, L], f32, tag="lnw", name=f"lnw_{g}")
                rsq = tp.tile([128, L], bf16, tag="zr", name=f"rsq_{g}")
                for lc in range(2):
                    cs = slice(lc * 512, (lc + 1) * 512)
                    sq_ps = psU.tile([128, 512], f32, tag="u",
                                     name=f"sq_ps_{g}_{lc}")
                    with nc.allow_low_precision(reason="bf16 z^2 RMS sum"):
                        nc.vector.tensor_mul(zsq[:, cs], z[:, cs], z[:, cs])
                    nc.tensor.matmul(
                        sq_ps[:], ones[:], zsq[:, cs],
                        start=True, stop=True,
                    )
                    # lnw = ln(sq/128 + eps*(S0*S1)^2)  (f32: rsq is exp of
                    # -lnw/2, so lnw needs absolute accuracy ~1e-3)
                    nc.vector.scalar_tensor_tensor(
                        out=lnw[:, cs], in0=sq_ps[:], scalar=1.0 / 128,
                        in1=wt[:, cs], op0=ALU.mult, op1=ALU.add,
                    )
                    nc.scalar.activation(
                        out=lnw[:, cs], in_=lnw[:, cs], func=AF.Ln,
                    )
                    with nc.allow_low_precision(reason="bf16 rsqrt bcast"):
                        nc.scalar.activation(
                            out=rsq[:, cs], in_=lnw[:, cs], func=AF.Exp,
                            scale=-0.5,
                        )
                        nc.vector.tensor_mul(
                            outT[:, g, cs], z[:, cs], rsq[:, cs]
                        )

            # ------------- interleaved emission schedule -------------
            # scores/exp for group g pace each loop; qkT/v/u units slot in
            # between the kb steps so the PE stream stays dense.

            emit_qk0_head(0)

            es_all = {}
            t4_all = {}
            sums_all = {}
            z_all = {}
            u_tiles = {}

            es_full_all = {}

            def new_g_state(g):
                es_full = ep.tile([128, 8, 2 * L], bf16, tag="e",
                                  name=f"e_{g}")
                es_full_all[g] = es_full
                es_all[g] = [es_full[:, :, 0:L], es_full[:, :, L:2 * L]]
                t4_all[g] = [
                    t4p.tile([128, 4, L], bf16, tag="t40", name=f"t40_{g}"),
                    t4p.tile([128, 4, L], bf16, tag="t41", name=f"t41_{g}"),
                ]

            def pull_u(g, lc, s):
                u_tiles[(g, lc, s)] = unit_u_half(g, lc, s, es_all[g])
                if s == 1 and g < 3:
                    emit_combine(
                        g, lc,
                        [u_tiles[(g, lc, 0)], u_tiles[(g, lc, 1)]],
                        sums_all[g], z_all[g],
                    )

            def u3_start(lc, s):
                # u(3) partial accumulation over the k-chunks whose exps are
                # already emitted; the group resumes with kb 6..7 after the
                # score loop (has_written state persists in the bank)
                cs = slice(lc * 512, (lc + 1) * 512)
                pool = psA if lc == 1 else psU
                u = pool.tile([128, 512], f32,
                              tag="acc" if pool is psA else "u",
                              name=f"u3p_{lc}_{s}")
                u_tiles[(3, lc, s)] = u
                for kb in range(6):
                    nc.tensor.matmul(
                        u[:], v[:, kb, 384:512], es_all[3][s][:, kb, cs],
                        start=(kb == 0), stop=False,
                    )

            def u3_finish(lc, s):
                cs = slice(lc * 512, (lc + 1) * 512)
                u = u_tiles[(3, lc, s)]
                for kb in (6, 7):
                    nc.tensor.matmul(
                        u[:], v[:, kb, 384:512], es_all[3][s][:, kb, cs],
                        start=False, stop=(kb == 7),
                    )

            # background unit schedule, one ~8-matmul pull per score
            # iteration so the PE stream stays dense past the exp pace
            pulls = {
                0: [lambda: unit_qk(1, "q", 0), lambda: unit_qk(1, "k", 0),
                    lambda: unit_qk(1, "q", 1), lambda: unit_qk(1, "k", 1),
                    lambda: unit_v(0), lambda: unit_v(1),
                    lambda: unit_v(2), lambda: unit_v(3)],
                1: [lambda: unit_v(4), lambda: unit_v(5),
                    lambda: unit_v(6), lambda: unit_v(7),
                    lambda: pull_u(0, 0, 0), lambda: pull_u(0, 0, 1),
                    lambda: pull_u(0, 1, 0), lambda: pull_u(0, 1, 1)],
                2: [lambda: pull_u(1, 0, 0), lambda: pull_u(1, 0, 1),
                    lambda: pull_u(1, 1, 0), lambda: pull_u(1, 1, 1),
                    lambda: (emit_tail(0, z_all[0], sums_all[0]),
                             unit_qk(3, "q", 0)),
                    lambda: (unit_qk(3, "k", 0),
                             emit_tail(1, z_all[1], sums_all[1])),
                    lambda: unit_qk(3, "q", 1), lambda: unit_qk(3, "k", 1)],
                3: [lambda: pull_u(2, 0, 0), lambda: pull_u(2, 0, 1),
                    lambda: pull_u(2, 1, 0), lambda: pull_u(2, 1, 1),
                    lambda: emit_tail(2, z_all[2], sums_all[2]),
                    lambda: u3_start(0, 0), lambda: u3_start(0, 1),
                    lambda: u3_start(1, 0)],
            }

            for g in range(4):
                new_g_state(g)
                z_all[g] = tp.tile([128, L], bf16, tag="z", name=f"z_{g}")
                es = es_all[g]
                for kb in range(8):
                    emit_scores_kb(g, kb, es_full_all[g])
                    if kb % 2 == 1:
                        emit_lvl1(g, es, t4_all[g], 0, kb // 2)
                        emit_lvl1(g, es, t4_all[g], 1, kb // 2)
                    pull = pulls[g][kb]
                    if pull is not None:
                        pull()
                if g == 3:
                    # finish the partially-accumulated u(3) groups now that
                    # the trailing exps exist, and run the last full half
                    u3_finish(0, 0)
                    u3_finish(0, 1)
                    u3_finish(1, 0)
                    u_tiles[(3, 1, 1)] = unit_u_half(3, 1, 1, es_all[3])
                sums_all[g] = emit_sums(g, t4_all[g])
                if g == 1:
                    # qkT for group 2 must precede scores(2) on the PE
                    unit_qk(2, "q", 0)
                    unit_qk(2, "k", 0)
                    unit_qk(2, "q", 1)
                    unit_qk(2, "k", 1)

            # ---------------- drain + output projection ----------------
            # Stage D y-tiles borrow the score-tag [128, 2048] PSUM tiles
            # (4 y accumulators per alloc) inside the main block, so there
            # is no pool barrier: the g=0..2 accumulations stream during
            # the combine/tail DVE chain and the g=3 row lands per tile as
            # soon as outT[:, 3] is written.
            y_r = y_d.rearrange("(lb p) n -> p lb n", p=128)
            tiles = [(lb, nk) for lb in range(8) for nk in range(2)]

            ycount = [0]

            def y_mm(acc_slice, ti, ggs):
                lb, nk = tiles[ti]
                for gg in ggs:
                    nc.tensor.matmul(
                        acc_slice,
                        outT[:, gg, lb * 128:(lb + 1) * 128],
                        wo_s[:, gg, nk * 512:(nk + 1) * 512],
                        start=(gg == 0),
                        stop=(gg == 3),
                    )

            def y_evac_one(acc_slice, ti):
                i = ycount[0]
                ycount[0] += 1
                lb, nk = tiles[ti]
                yt = tp.tile([128, 512], f32, tag="yt", name=f"yt_{ti}")
                if i % 2 == 0:
                    nc.scalar.copy(out=yt[:], in_=acc_slice)
                else:
                    nc.vector.tensor_copy(yt[:], acc_slice)
                q = nc.sync if i % 2 == 0 else nc.gpsimd
                q.dma_start(y_r[:, lb, nk * 512:(nk + 1) * 512], yt[:])

            # drain group 3: eps term first (needs only sums), then the two
            # combine halves; y tiles 0-5 run their g0..2 accumulation
            # during the combine/tail DVE chain (score-PSUM + psA borrows),
            # finish with the g=3 row once outT[:, 3] lands, and the
            # remaining tiles stream behind them.
            ctwt3 = emit_tail_ctwt(3, sums_all[3])
            yacc0 = psS.tile([128, 2 * L], f32, tag="sc", bufs=1,
                             name="yacc0")
            for j in range(4):
                y_mm(yacc0[:, j * 512:(j + 1) * 512], j, range(3))
            emit_combine(3, 0, [u_tiles[(3, 0, 0)], u_tiles[(3, 0, 1)]],
                         sums_all[3], z_all[3])
            emit_combine(3, 1, [u_tiles[(3, 1, 0)], u_tiles[(3, 1, 1)]],
                         sums_all[3], z_all[3])
            ya4 = psA.tile([128, 512], f32, tag="acc", name="ya4")
            ya5 = psA.tile([128, 512], f32, tag="acc", name="ya5")
            y_mm(ya4[:], 4, range(3))
            y_mm(ya5[:], 5, range(3))
            emit_tail(3, z_all[3], sums_all[3], ctwt=ctwt3)
            # g=3 contributions + evacuation for the six early tiles
            for j in range(4):
                y_mm(yacc0[:, j * 512:(j + 1) * 512], j, (3,))
            for j in range(4):
                y_evac_one(yacc0[:, j * 512:(j + 1) * 512], j)
            y_mm(ya4[:], 4, (3,))
            y_mm(ya5[:], 5, (3,))
            y_evac_one(ya4[:], 4)
            y_evac_one(ya5[:], 5)
            # remaining tiles: two full [128,2048] groups + a psU pair
            for i4, base in ((1, 6), (2, 10)):
                acc = psS.tile([128, 2 * L], f32, tag="sc", bufs=1,
                               name=f"yacc_{i4}")
                for j in range(4):
                    y_mm(acc[:, j * 512:(j + 1) * 512], base + j, range(4))
                for j in range(4):
                    y_evac_one(acc[:, j * 512:(j + 1) * 512], base + j)
            yu14 = psU.tile([128, 512], f32, tag="u", name="yu14")
            yu15 = psU.tile([128, 512], f32, tag="u", name="yu15")
            y_mm(yu14[:], 14, range(4))
            y_mm(yu15[:], 15, range(4))
            y_evac_one(yu14[:], 14)
            y_evac_one(yu15[:], 15)

        persist.release()
    if split_waits:
        _split_excess_waits(nc)
    return nc


def kernel(**inputs) -> np.ndarray:
    from concourse.bass_utils import run_bass_kernel_spmd

    bf = ml_dtypes.bfloat16
    q_in = np.asarray(inputs["query"], np.float32)      # (L, NB, D)
    Wq = np.asarray(inputs["Wq"], np.float32)
    Wk = np.asarray(inputs["Wk"], np.float32)
    Wv = np.asarray(inputs["Wv"], np.float32)
    Wo = np.asarray(inputs["Wo"], np.float32)
    bq = np.asarray(inputs["bq"], np.float32)
    bk = np.asarray(inputs["bk"], np.float32)
    bv = np.asarray(inputs["bv"], np.float32)
    bo = np.asarray(inputs["bo"], np.float32)
    norm_w = np.asarray(inputs["norm_w"], np.float32)
    mask = np.asarray(inputs["attn_mask"], np.float32)
    lq1 = np.asarray(inputs["lq1"], np.float32)
    lk1 = np.asarray(inputs["lk1"], np.float32)
    lq2 = np.asarray(inputs["lq2"], np.float32)
    lk2 = np.asarray(inputs["lk2"], np.float32)

    lam = float(
        np.exp(np.sum(lq1 * lk1)) - np.exp(np.sum(lq2 * lk2)) + LAMBDA_INIT
    )
    scale = HD ** -0.5
    with_mask = bool(np.any(mask))
    with_qk_bias = bool(np.any(bq) or np.any(bk))
    with_v_bias = bool(np.any(bv))
    # norm_w * (1 - lambda_init) folded into Wo rows (tiled per he-head)
    nw = np.tile(norm_w * (1.0 - LAMBDA_INIT), HE // 2)  # (COLS,)

    nc = _build(lam, with_mask, with_qk_bias, with_v_bias)

    maskT = np.ascontiguousarray(mask.T).astype(bf) if with_mask else None
    in_maps = []
    for c in range(NCORES):
        b, g2 = divmod(c, 2)
        cols = slice(COLS * g2, COLS * (g2 + 1))
        x = q_in[:, b, :]
        im = {
            "xT": np.ascontiguousarray(x.T).astype(bf),
            "wq": (Wq[:, cols] * scale).astype(bf),
            "wk": np.ascontiguousarray(Wk[:, cols]).astype(bf),
            "wv": np.ascontiguousarray(Wv[:, cols]).astype(bf),
            "wo": (Wo[cols, :] * nw[:, None]).astype(bf),
        }
        if with_qk_bias:
            im["bqs"] = np.ascontiguousarray(bq[cols] * scale)
            im["bks"] = np.ascontiguousarray(bk[cols])
        if with_v_bias:
            im["bvs"] = np.ascontiguousarray(bv[cols])
        if with_mask:
            im["maskT"] = maskT
        in_maps.append(im)

    res = run_bass_kernel_spmd(nc, in_maps, core_ids=list(range(NCORES)))
    global LAST_RESULT
    LAST_RESULT = res
    outs = [r["y"] for r in res.results]

    out = np.empty((L, NB, D), np.float32)
    for b in range(NB):
        yb = outs[2 * b] + outs[2 * b + 1]
        if np.any(bo):
            yb = yb + bo
        out[:, b, :] = yb
    return out


# revision 41
# speedup vs baseline: 1.0466x; 1.0466x over previous
"""DiffCLIP differential-attention block on 8 Trainium2 NeuronCores, v3.

Sharding: the (batch=4) x (head-group=2) grid maps to the 8 cores — core
c = 2*b + g handles batch b and half the heads (4 of 8 effective heads),
i.e. a 512-column slice of the q/k/v projections and the matching 512-row
slice of the out projection. Each core emits a partial (L, D) output; the
host sums the two per-batch partials and stacks.

v3 changes vs v2:
  - K=64 score matmuls emitted as adjacent row-group pairs
    (tile_position (0,0)/(64,0)) so the PE runs both concurrently
  - RMS tail is DRAM-round-trip-free: rsqrt computed as exp(-0.5*ln(w))
    on partition-replicated [128, L] tiles — keeps ScalarE on the single
    natural_log_exp_and_others table set (no ~2.7us table reloads)
  - fine-grained software pipeline: qkT/v/u matmul "units" interleaved
    into the score/exp stream so the PE never idles long enough for the
    HAM clock gate to re-throttle
  - out-projection starts its g=0..2 PSUM accumulation while the last
    pair's tail still computes; y evacuations split across ACT and DVE
  - weight DMAs ordered so the first matmul's operands arrive first
"""

import sys

if "/opt/trn_rl_repo" not in sys.path:
    sys.path.insert(0, "/opt/trn_rl_repo")

import numpy as np
import ml_dtypes

L, D, H, HD, HE = 1024, 1024, 16, 64, 8
LAMBDA_INIT = 0.8
EPS = 1e-5
NB = 4
NCORES = 8
COLS = 512  # per-core projection column count

LAST_RESULT = None  # BassKernelResults of the most recent kernel() call


def _split_excess_waits(nc, max_waits: int = 1):
    """Walrus codegen on this toolchain accepts at most one sync-wait command
    per hardware instruction (plus its update); Tile freely emits several.
    Split the excess waits onto preceding same-engine NoOps."""
    import bass_rust
    import concourse.mybir as mybir

    for f in nc.m.functions:
        for blk in f.blocks:
            insts = blk.instructions
            out = []
            changed = False
            for inst in insts:
                si = inst.sync_info
                if si is not None and si.on_wait and len(si.on_wait) > max_waits:
                    waits = list(si.on_wait)
                    for j, w in enumerate(waits[max_waits:]):
                        nop = mybir.InstNoOp(
                            name=f"{inst.name}-xw{j}",
                            sync_info=bass_rust.SyncInfo(
                                on_wait=[w], on_update=[]
                            ),
                            bass_nofuse=True,
                            engine=inst.engine,
                        )
                        nc.register_instruction(nop, overwrite=True)
                        out.append(nop)
                    inst.sync_info = bass_rust.SyncInfo(
                        on_wait=waits[:max_waits],
                        on_update=list(si.on_update or []),
                    )
                    changed = True
                out.append(inst)
            if changed:
                blk.instructions = out


def _build(lam: float, with_mask: bool, with_qk_bias: bool, with_v_bias: bool,
           split_waits: bool = True):
    import concourse.bass as bass
    import concourse.tile as tile
    import concourse.mybir as mybir

    bf16 = mybir.dt.bfloat16
    f32 = mybir.dt.float32
    AF = mybir.ActivationFunctionType
    ALU = mybir.AluOpType

    SQRT_EPS = float(np.sqrt(EPS))

    nc = bass.Bass()
    xT_d = nc.dram_tensor("xT", [D, L], bf16, kind="ExternalInput")
    wq_d = nc.dram_tensor("wq", [D, COLS], bf16, kind="ExternalInput")
    wk_d = nc.dram_tensor("wk", [D, COLS], bf16, kind="ExternalInput")
    wv_d = nc.dram_tensor("wv", [D, COLS], bf16, kind="ExternalInput")
    wo_d = nc.dram_tensor("wo", [COLS, D], bf16, kind="ExternalInput")
    if with_qk_bias:
        bq_d = nc.dram_tensor("bqs", [COLS], f32, kind="ExternalInput")
        bk_d = nc.dram_tensor("bks", [COLS], f32, kind="ExternalInput")
    if with_v_bias:
        bv_d = nc.dram_tensor("bvs", [COLS], f32, kind="ExternalInput")
    if with_mask:
        maskT_d = nc.dram_tensor("maskT", [L, L], bf16, kind="ExternalInput")
    y_d = nc.dram_tensor("y", [L, D], f32, kind="ExternalOutput")

    with tile.TileContext(nc) as tc:
        persist = tc.alloc_tile_pool(name="persist", bufs=1)
        qT = persist.tile([128, 4, L], bf16)
        kT = persist.tile([128, 4, L], bf16)
        v = persist.tile([128, 8, COLS], bf16)
        wo_s = persist.tile([128, 4, D], bf16)
        outT = persist.tile([128, 4, L], bf16)
        ones = persist.tile([128, 128], bf16)
        nc.vector.memset(ones, 1.0)
        if with_mask:
            ident = persist.tile([128, 128], bf16)
            from concourse.masks import make_identity
            make_identity(nc, ident)
            maskT_s = persist.tile([128, 8, L], bf16)
            nc.sync.dma_start(
                maskT_s, maskT_d.rearrange("(ko p) l -> p ko l", p=128)
            )

        with (
            tc.tile_pool(name="stage_a", bufs=1) as sa,
            tc.tile_pool(name="epool", bufs=2) as ep,
            tc.tile_pool(name="sums", bufs=2) as sp,
            tc.tile_pool(name="t4p", bufs=1) as t4p,
            tc.tile_pool(name="tailp", bufs=2) as tp,
            tc.tile_pool(name="psA", bufs=2, space="PSUM") as psA,
            tc.tile_pool(name="psS", bufs=2, space="PSUM") as psS,
            tc.tile_pool(name="psU", bufs=2, space="PSUM") as psU,
        ):
            xts = sa.tile([128, 8, L], bf16)
            wq_s = sa.tile([128, 8, COLS], bf16)
            wk_s = sa.tile([128, 8, COLS], bf16)
            wv_s = sa.tile([128, 8, COLS], bf16)
            xT_r = xT_d.rearrange("(ko p) l -> p ko l", p=128)
            wq_r = wq_d.rearrange("(ko p) m -> p ko m", p=128)
            wk_r = wk_d.rearrange("(ko p) m -> p ko m", p=128)
            wv_r = wv_d.rearrange("(ko p) m -> p ko m", p=128)
            # x split across sync+gpsimd queues; g0 slices of wq/wk first on
            # scalar/vector queues so the first matmuls' operands land early.
            # x spread over all three DMA-capable queues, g0 weight slices
            # first so the opening qkT units are never transfer-starved
            nc.scalar.dma_start(wq_s[:, :, 0:128], wq_r[:, :, 0:128])
            for kb in range(3):
                nc.sync.dma_start(xts[:, kb], xT_r[:, kb])
            for kb in range(3, 6):
                nc.gpsimd.dma_start(xts[:, kb], xT_r[:, kb])
            nc.scalar.dma_start(wk_s[:, :, 0:128], wk_r[:, :, 0:128])
            nc.scalar.dma_start(xts[:, 6], xT_r[:, 6])
            nc.scalar.dma_start(xts[:, 7], xT_r[:, 7])
            nc.sync.dma_start(wq_s[:, :, 128:512], wq_r[:, :, 128:512])
            nc.gpsimd.dma_start(wk_s[:, :, 128:512], wk_r[:, :, 128:512])
            nc.gpsimd.dma_start(wv_s[:], wv_r[:])
            nc.sync.dma_start(wo_s, wo_d.rearrange("(ko p) n -> p ko n", p=128))
            if with_qk_bias:
                bq_s = sa.tile([128, 4], f32)
                bk_s = sa.tile([128, 4], f32)
                nc.sync.dma_start(bq_s, bq_d.rearrange("(mb p) -> p mb", p=128))
                nc.sync.dma_start(bk_s, bk_d.rearrange("(mb p) -> p mb", p=128))
            if with_v_bias:
                bv_s = sa.tile([128, COLS], f32)
                bv_ap = bv_d[:]
                nc.gpsimd.dma_start(
                    bv_s,
                    bass.AP(
                        tensor=bv_ap.tensor,
                        offset=bv_ap.offset,
                        ap=[[0, 128], list(bv_ap.ap[0])],
                    ),
                )

            # ------------- emission building blocks -------------

            def evac_qk(which, g, lc, acc):
                dst = qT if which == "q" else kT
                dst_ap = dst[:, g, lc * 512:(lc + 1) * 512]
                if with_qk_bias:
                    b_s = bq_s if which == "q" else bk_s
                    nc.scalar.activation(
                        out=dst_ap, in_=acc[:], func=AF.Identity,
                        bias=b_s[:, g:g + 1], scale=1.0,
                    )
                else:
                    nc.vector.tensor_copy(dst_ap, acc[:])

            def unit_qk(g, which, lc):
                # one [128 ch, 512 tok] slice of qT/kT for group g
                wt_s = wq_s if which == "q" else wk_s
                acc = psA.tile([128, 512], f32, tag="acc")
                for kb in range(8):
                    nc.tensor.matmul(
                        acc[:],
                        wt_s[:, kb, g * 128:(g + 1) * 128],
                        xts[:, kb, lc * 512:(lc + 1) * 512],
                        start=(kb == 0),
                        stop=(kb == 7),
                    )
                evac_qk(which, g, lc, acc)

            def emit_qk0_head(g):
                # opening qkT in k-chunk-major order across four parallel
                # accumulators (borrowing the idle psU slots) so each
                # arriving x chunk feeds four matmuls immediately — hides
                # the x transfer tail behind compute
                slots = [("q", 0, psA), ("k", 0, psA), ("q", 1, psU),
                         ("k", 1, psU)]
                accs = []
                for which, lc, pool in slots:
                    accs.append(pool.tile(
                        [128, 512], f32,
                        tag="acc" if pool is psA else "u",
                        name=f"qk0_{which}_{lc}"))
                for kb in range(8):
                    for (which, lc, pool), acc in zip(slots, accs):
                        wt_s = wq_s if which == "q" else wk_s
                        nc.tensor.matmul(
                            acc[:],
                            wt_s[:, kb, g * 128:(g + 1) * 128],
                            xts[:, kb, lc * 512:(lc + 1) * 512],
                            start=(kb == 0),
                            stop=(kb == 7),
                        )
                for (which, lc, pool), acc in zip(slots, accs):
                    evac_qk(which, g, lc, acc)

            def unit_v(lb):
                # v rows [128 tok chunk lb, 512 ch]
                acc = psA.tile([128, 512], f32, tag="acc")
                for kb in range(8):
                    nc.tensor.matmul(
                        acc[:],
                        xts[:, kb, lb * 128:(lb + 1) * 128],
                        wv_s[:, kb, :],
                        start=(kb == 0),
                        stop=(kb == 7),
                    )
                if with_v_bias:
                    nc.vector.tensor_add(v[:, lb, :], acc[:], bv_s[:])
                else:
                    nc.vector.tensor_copy(v[:, lb, :], acc[:])

            def emit_score_mm(g, s, kb, lc, sc):
                nc.tensor.matmul(
                    sc[:, lc * 512:(lc + 1) * 512],
                    kT[64 * s:64 * (s + 1), g, kb * 128:(kb + 1) * 128],
                    qT[64 * s:64 * (s + 1), g, lc * 512:(lc + 1) * 512],
                    start=True,
                    stop=not with_mask,
                    tile_position=(64 * s, 0),
                )
                if with_mask:
                    nc.tensor.matmul(
                        sc[:, lc * 512:(lc + 1) * 512],
                        ident[:],
                        maskT_s[:, kb, lc * 512:(lc + 1) * 512],
                        start=False,
                        stop=True,
                    )

            def emit_scores_kb(g, kb, es_full):
                # all four K=64 score matmuls of this k-chunk land in ONE
                # 4-bank PSUM tile: only the quartet's first matmul carries
                # the tile's drain wait, so the following row-group-disjoint
                # partners issue wait-free and the PE packs each pair
                # concurrently (observed: a waiting matmul never co-issues).
                sc = psS.tile([128, 2 * L], f32, tag="sc", bufs=1,
                              name=f"sc_{g}_{kb}")
                for lc in range(2):
                    for s in range(2):
                        emit_score_mm(g, s, kb, lc, sc[:, s * L:(s + 1) * L])
                # ONE exp drains the whole 4-bank tile: the next chunk's
                # quartet then waits on a single older event, and the 2048
                # free-dim amortizes the ACT fixed overhead
                nc.scalar.activation(
                    out=es_full[:, kb, :], in_=sc[:], func=AF.Exp,
                )

            def emit_lvl1(g, es, t4s, s, j):
                # first-level pair add of the row-sum tree (bf16, 2x DVE)
                nc.vector.tensor_add(
                    t4s[s][:, j], es[s][:, 2 * j], es[s][:, 2 * j + 1]
                )

            def emit_sums(g, t4s):
                # finish tree in-place, then ones-matmul partition reduction
                # leaves row sums replicated across partitions. The copies
                # fold in a sqrt(eps) scale: S' = sqrt(eps)*S makes the RMS
                # eps-term simply (S0'*S1')^2 (the S scale itself cancels
                # between z and the denominator), and bf16 S is safe for the
                # same cancellation reason.
                srep_ps = psS.tile([128, 2 * L], f32, tag="sc", bufs=1,
                                   name=f"srep_ps_{g}")
                reps = []
                for s in range(2):
                    t4 = t4s[s]
                    nc.vector.tensor_add(t4[:, 0], t4[:, 0], t4[:, 1])
                    nc.vector.tensor_add(t4[:, 2], t4[:, 2], t4[:, 3])
                    nc.vector.tensor_add(t4[:, 0], t4[:, 0], t4[:, 2])
                    for lc in range(2):
                        nc.tensor.matmul(
                            srep_ps[:, s * L + lc * 512:s * L + (lc + 1) * 512],
                            ones[:],
                            t4[:, 0, lc * 512:(lc + 1) * 512],
                            start=True,
                            stop=True,
                        )
                    srep = sp.tile([128, L], bf16, tag=f"s{s}",
                                   name=f"srep_{g}_{s}")
                    with nc.allow_low_precision(reason="S scale cancels"):
                        nc.scalar.activation(
                            out=srep[:], in_=srep_ps[:, s * L:(s + 1) * L],
                            func=AF.Identity, scale=SQRT_EPS,
                        )
                    reps.append(srep)
                return reps

            def unit_u_half(g, lc, s, es):
                # u_s = v^T e_s for one 512-token q chunk, one s stream
                cs = slice(lc * 512, (lc + 1) * 512)
                pool = psA if (g == 3 and lc == 1) else psU
                u = pool.tile([128, 512], f32,
                              tag="acc" if pool is psA else "u",
                              name=f"u_{g}_{lc}_{s}")
                for kb in range(8):
                    nc.tensor.matmul(
                        u[:],
                        v[:, kb, 128 * g:128 * (g + 1)],
                        es[s][:, kb, cs],
                        start=(kb == 0),
                        stop=(kb == 7),
                    )
                return u

            def emit_combine(g, lc, us, reps, z):
                cs = slice(lc * 512, (lc + 1) * 512)
                s0_rep, s1_rep = reps
                tt = tp.tile([128, 512], bf16, tag="tt", name=f"tt_{g}_{lc}")
                with nc.allow_low_precision(reason="bf16 z, scale cancels"):
                    nc.vector.tensor_mul(z[:, cs], us[0][:], s1_rep[:, cs])
                    nc.vector.scalar_tensor_tensor(
                        out=tt[:], in0=us[1][:], scalar=lam,
                        in1=s0_rep[:, cs], op0=ALU.mult, op1=ALU.mult,
                    )
                    nc.vector.tensor_sub(z[:, cs], z[:, cs], tt[:])

            def emit_tail_ctwt(g, reps):
                # (S0'*S1')^2 = eps*(S0*S1)^2 on replicated tiles (the
                # sqrt(eps) scale was folded into the S copies); depends
                # only on sums, so it can be emitted before the combines.
                s0_rep, s1_rep = reps
                ct = tp.tile([128, L], bf16, tag="ct", name=f"ct_{g}")
                wt = tp.tile([128, L], bf16, tag="wt", name=f"wt_{g}")
                with nc.allow_low_precision(reason="eps term only"):
                    nc.vector.tensor_mul(ct[:], s1_rep[:], s0_rep[:])
                    nc.vector.tensor_mul(wt[:], ct[:], ct[:])
                return ct, wt

            def emit_tail(g, z, reps, ctwt=None):
                # headwise RMS over the 128-partition channel dim, all on
                # replicated [128, L] tiles: out = z * rsqrt(sum(z^2)/128
                # + eps*(S0*S1)^2), with rsqrt = exp(-0.5*ln(.)) so ScalarE
                # stays on the natural_log_exp table set. The z^2 partition
                # sums go through psU tiles so the score-PSUM banks drain
                # early for the output projection's early start.
                if ctwt is None:
                    ctwt = emit_tail_ctwt(g, reps)
                ct, wt = ctwt
                zsq = tp.tile([128, L], bf16, tag="zr", name=f"zsq_{g}")
                lnw = tp.tile([128, L], f32, tag="lnw", name=f"lnw_{g}")
                rsq = tp.tile([128, L], bf16, tag="zr", name=f"rsq_{g}")
                for lc in range(2):
                    cs = slice(lc * 512, (lc + 1) * 512)
                    sq_ps = psU.tile([128, 512], f32, tag="u",
                                     name=f"sq_ps_{g}_{lc}")
                    with nc.allow_low_precision(reason="bf16 z^2 RMS sum"):
                        nc.vector.tensor_mul(zsq[:, cs], z[:, cs], z[:, cs])
                    nc.tensor.matmul(
                        sq_ps[:], ones[:], zsq[:, cs],
                        start=True, stop=True,
                    )
                    # lnw = ln(sq/128 + eps*(S0*S1)^2)  (f32: rsq is exp of
                    # -lnw/2, so lnw needs absolute accuracy ~1e-3)
                    nc.vector.scalar_tensor_tensor(
                        out=lnw[:, cs], in0=sq_ps[:], scalar=1.0 / 128,
                        in1=wt[:, cs], op0=ALU.mult, op1=ALU.add,
                    )
                    nc.scalar.activation(
                        out=lnw[:, cs], in_=lnw[:, cs], func=AF.Ln,
                    )
                    with nc.allow_low_precision(reason="bf16 rsqrt bcast"):
                        nc.scalar.activation(
                            out=rsq[:, cs], in_=lnw[:, cs], func=AF.Exp,
                            scale=-0.5,
                        )
                        nc.vector.tensor_mul(
                            outT[:, g, cs], z[:, cs], rsq[:, cs]
                        )

            # ------------- interleaved emission schedule -------------
            # scores/exp for group g pace each loop; qkT/v/u units slot in
            # between the kb steps so the PE stream stays dense.

            emit_qk0_head(0)

            es_all = {}
            t4_all = {}
            sums_all = {}
            z_all = {}
            u_tiles = {}

            es_full_all = {}

            def new_g_state(g):
                es_full = ep.tile([128, 8, 2 * L], bf16, tag="e",
                                  name=f"e_{g}")
                es_full_all[g] = es_full
                es_all[g] = [es_full[:, :, 0:L], es_full[:, :, L:2 * L]]
                t4_all[g] = [
                    t4p.tile([128, 4, L], bf16, tag="t40", name=f"t40_{g}"),
                    t4p.tile([128, 4, L], bf16, tag="t41", name=f"t41_{g}"),
                ]

            def pull_u(g, lc, s):
                u_tiles[(g, lc, s)] = unit_u_half(g, lc, s, es_all[g])
                if s == 1 and g < 3:
                    emit_combine(
                        g, lc,
                        [u_tiles[(g, lc, 0)], u_tiles[(g, lc, 1)]],
                        sums_all[g], z_all[g],
                    )

            def u3_start(lc, s):
                # u(3) partial accumulation over the k-chunks whose exps are
                # already emitted; the group resumes with kb 6..7 after the
                # score loop (has_written state persists in the bank)
                cs = slice(lc * 512, (lc + 1) * 512)
                pool = psA if lc == 1 else psU
                u = pool.tile([128, 512], f32,
                              tag="acc" if pool is psA else "u",
                              name=f"u3p_{lc}_{s}")
                u_tiles[(3, lc, s)] = u
                for kb in range(6):
                    nc.tensor.matmul(
                        u[:], v[:, kb, 384:512], es_all[3][s][:, kb, cs],
                        start=(kb == 0), stop=False,
                    )

            def u3_finish(lc, s):
                cs = slice(lc * 512, (lc + 1) * 512)
                u = u_tiles[(3, lc, s)]
                for kb in (6, 7):
                    nc.tensor.matmul(
                        u[:], v[:, kb, 384:512], es_all[3][s][:, kb, cs],
                        start=False, stop=(kb == 7),
                    )

            # background unit schedule, one ~8-matmul pull per score
            # iteration so the PE stream stays dense past the exp pace
            pulls = {
                0: [lambda: unit_qk(1, "q", 0), lambda: unit_qk(1, "k", 0),
                    lambda: unit_qk(1, "q", 1), lambda: unit_qk(1, "k", 1),
                    lambda: unit_v(0), lambda: unit_v(1),
                    lambda: unit_v(2), lambda: unit_v(3)],
                1: [lambda: unit_v(4), lambda: unit_v(5),
                    lambda: unit_v(6), lambda: unit_v(7),
                    lambda: pull_u(0, 0, 0), lambda: pull_u(0, 0, 1),
                    lambda: pull_u(0, 1, 0), lambda: pull_u(0, 1, 1)],
                2: [lambda: pull_u(1, 0, 0), lambda: pull_u(1, 0, 1),
                    lambda: pull_u(1, 1, 0), lambda: pull_u(1, 1, 1),
                    lambda: (emit_tail(0, z_all[0], sums_all[0]),
                             unit_qk(3, "q", 0)),
                    lambda: (unit_qk(3, "k", 0),
                             emit_tail(1, z_all[1], sums_all[1])),
                    lambda: unit_qk(3, "q", 1), lambda: unit_qk(3, "k", 1)],
                3: [lambda: pull_u(2, 0, 0), lambda: pull_u(2, 0, 1),
                    lambda: pull_u(2, 1, 0), lambda: pull_u(2, 1, 1),
                    lambda: emit_tail(2, z_all[2], sums_all[2]),
                    lambda: u3_start(0, 0), lambda: u3_start(0, 1),
                    lambda: u3_start(1, 0)],
            }

            for g in range(4):
                new_g_state(g)
                z_all[g] = tp.tile([128, L], bf16, tag="z", name=f"z_{g}")
                es = es_all[g]
                for kb in range(8):
                    emit_scores_kb(g, kb, es_full_all[g])
                    if kb % 2 == 1:
                        emit_lvl1(g, es, t4_all[g], 0, kb // 2)
                        emit_lvl1(g, es, t4_all[g], 1, kb // 2)
                    pull = pulls[g][kb]
                    if pull is not None:
                        pull()
                if g == 3:
                    # finish the partially-accumulated u(3) groups now that
                    # the trailing exps exist, and run the last full half
                    u3_finish(0, 0)
                    u3_finish(0, 1)
                    u3_finish(1, 0)
                    u_tiles[(3, 1, 1)] = unit_u_half(3, 1, 1, es_all[3])
                sums_all[g] = emit_sums(g, t4_all[g])
                if g == 1:
                    # qkT for group 2 must precede scores(2) on the PE
                    unit_qk(2, "q", 0)
                    unit_qk(2, "k", 0)
                    unit_qk(2, "q", 1)
                    unit_qk(2, "k", 1)

            # ---------------- drain + output projection ----------------
            # Stage D y-tiles borrow the score-tag [128, 2048] PSUM tiles
            # (4 y accumulators per alloc) inside the main block, so there
            # is no pool barrier: the g=0..2 accumulations stream during
            # the combine/tail DVE chain and the g=3 row lands per tile as
            # soon as outT[:, 3] is written.
            y_r = y_d.rearrange("(lb p) n -> p lb n", p=128)
            tiles = [(lb, nk) for lb in range(8) for nk in range(2)]

            # drain group 3: eps term first (needs only sums), then the two
            # combine halves feeding the tail pipeline
            ctwt3 = emit_tail_ctwt(3, sums_all[3])
            emit_combine(3, 0, [u_tiles[(3, 0, 0)], u_tiles[(3, 0, 1)]],
                         sums_all[3], z_all[3])
            emit_combine(3, 1, [u_tiles[(3, 1, 0)], u_tiles[(3, 1, 1)]],
                         sums_all[3], z_all[3])
            emit_tail(3, z_all[3], sums_all[3], ctwt=ctwt3)

        # ---------------- Stage D: output projection ----------------
        # first tiles accumulate g=0..2 while tail(3) computes; the g=3
        # contribution and evacuation follow once outT[:, 3] lands.
        with (
            tc.tile_pool(name="yp", bufs=4) as yp,
            tc.tile_pool(name="psY", bufs=8, space="PSUM") as psY,
        ):
            y_r = y_d.rearrange("(lb p) n -> p lb n", p=128)
            tiles = [(lb, nk) for lb in range(8) for nk in range(2)]
            accs = {}
            for i, (lb, nk) in enumerate(tiles[:8]):
                acc = psY.tile([128, 512], f32, tag="y")
                accs[(lb, nk)] = acc
                for gg in range(3):
                    nc.tensor.matmul(
                        acc[:],
                        outT[:, gg, lb * 128:(lb + 1) * 128],
                        wo_s[:, gg, nk * 512:(nk + 1) * 512],
                        start=(gg == 0),
                        stop=False,
                    )
            for i, (lb, nk) in enumerate(tiles):
                if (lb, nk) in accs:
                    acc = accs[(lb, nk)]
                    nc.tensor.matmul(
                        acc[:],
                        outT[:, 3, lb * 128:(lb + 1) * 128],
                        wo_s[:, 3, nk * 512:(nk + 1) * 512],
                        start=False,
                        stop=True,
                    )
                else:
                    acc = psY.tile([128, 512], f32, tag="y")
                    for gg in range(4):
                        nc.tensor.matmul(
                            acc[:],
                            outT[:, gg, lb * 128:(lb + 1) * 128],
                            wo_s[:, gg, nk * 512:(nk + 1) * 512],
                            start=(gg == 0),
                            stop=(gg == 3),
                        )
                yt = yp.tile([128, 512], f32, tag="yt")
                if i % 2 == 0:
                    nc.scalar.copy(out=yt[:], in_=acc[:])
                else:
                    nc.vector.tensor_copy(yt[:], acc[:])
                q = nc.sync if i % 2 == 0 else nc.gpsimd
                q.dma_start(y_r[:, lb, nk * 512:(nk + 1) * 512], yt[:])

        persist.release()
    if split_waits:
        _split_excess_waits(nc)
    return nc


def kernel(**inputs) -> np.ndarray:
    from concourse.bass_utils import run_bass_kernel_spmd

    bf = ml_dtypes.bfloat16
    q_in = np.asarray(inputs["query"], np.float32)      # (L, NB, D)
    Wq = np.asarray(inputs["Wq"], np.float32)
    Wk = np.asarray(inputs["Wk"], np.float32)
    Wv = np.asarray(inputs["Wv"], np.float32)
    Wo = np.asarray(inputs["Wo"], np.float32)
    bq = np.asarray(inputs["bq"], np.float32)
    bk = np.asarray(inputs["bk"], np.float32)
    bv = np.asarray(inputs["bv"], np.float32)
    bo = np.asarray(inputs["bo"], np.float32)
    norm_w = np.asarray(inputs["norm_w"], np.float32)
    mask = np.asarray(inputs["attn_mask"], np.float32)
    lq1 = np.asarray(inputs["lq1"], np.float32)
    lk1 = np.asarray(inputs["lk1"], np.float32)
    lq2 = np.asarray(inputs["lq2"], np.float32)
    lk2 = np.asarray(inputs["lk2"], np.float32)

    lam = float(
        np.exp(np.sum(lq1 * lk1)) - np.exp(np.sum(lq2 * lk2)) + LAMBDA_INIT
    )
    scale = HD ** -0.5
    with_mask = bool(np.any(mask))
    with_qk_bias = bool(np.any(bq) or np.any(bk))
    with_v_bias = bool(np.any(bv))
    # norm_w * (1 - lambda_init) folded into Wo rows (tiled per he-head)
    nw = np.tile(norm_w * (1.0 - LAMBDA_INIT), HE // 2)  # (COLS,)

    nc = _build(lam, with_mask, with_qk_bias, with_v_bias)

    maskT = np.ascontiguousarray(mask.T).astype(bf) if with_mask else None
    in_maps = []
    for c in range(NCORES):
        b, g2 = divmod(c, 2)
        cols = slice(COLS * g2, COLS * (g2 + 1))
        x = q_in[:, b, :]
        im = {
            "xT": np.ascontiguousarray(x.T).astype(bf),
            "wq": (Wq[:, cols] * scale).astype(bf),
            "wk": np.ascontiguousarray(Wk[:, cols]).astype(bf),
            "wv": np.ascontiguousarray(Wv[:, cols]).astype(bf),
            "wo": (Wo[cols, :] * nw[:, None]).astype(bf),
        }
        if with_qk_bias:
            im["bqs"] = np.ascontiguousarray(bq[cols] * scale)
            im["bks"] = np.ascontiguousarray(bk[cols])
        if with_v_bias:
            im["bvs"] = np.ascontiguousarray(bv[cols])
        if with_mask:
            im["maskT"] = maskT
        in_maps.append(im)

    res = run_bass_kernel_spmd(nc, in_maps, core_ids=list(range(NCORES)))
    global LAST_RESULT
    LAST_RESULT = res
    outs = [r["y"] for r in res.results]

    out = np.empty((L, NB, D), np.float32)
    for b in range(NB):
        yb = outs[2 * b] + outs[2 * b + 1]
        if np.any(bo):
            yb = yb + bo
        out[:, b, :] = yb
    return out


# revision 46
# speedup vs baseline: 1.0700x; 1.0223x over previous
"""DiffCLIP differential-attention block on 8 Trainium2 NeuronCores, v3.

Sharding: the (batch=4) x (head-group=2) grid maps to the 8 cores — core
c = 2*b + g handles batch b and half the heads (4 of 8 effective heads),
i.e. a 512-column slice of the q/k/v projections and the matching 512-row
slice of the out projection. Each core emits a partial (L, D) output; the
host sums the two per-batch partials and stacks.

v3 changes vs v2:
  - K=64 score matmuls emitted as adjacent row-group pairs
    (tile_position (0,0)/(64,0)) so the PE runs both concurrently
  - RMS tail is DRAM-round-trip-free: rsqrt computed as exp(-0.5*ln(w))
    on partition-replicated [128, L] tiles — keeps ScalarE on the single
    natural_log_exp_and_others table set (no ~2.7us table reloads)
  - fine-grained software pipeline: qkT/v/u matmul "units" interleaved
    into the score/exp stream so the PE never idles long enough for the
    HAM clock gate to re-throttle
  - out-projection starts its g=0..2 PSUM accumulation while the last
    pair's tail still computes; y evacuations split across ACT and DVE
  - weight DMAs ordered so the first matmul's operands arrive first
"""

import sys

if "/opt/trn_rl_repo" not in sys.path:
    sys.path.insert(0, "/opt/trn_rl_repo")

import numpy as np
import ml_dtypes

L, D, H, HD, HE = 1024, 1024, 16, 64, 8
LAMBDA_INIT = 0.8
EPS = 1e-5
NB = 4
NCORES = 8
COLS = 512  # per-core projection column count

LAST_RESULT = None  # BassKernelResults of the most recent kernel() call


def _split_excess_waits(nc, max_waits: int = 1):
    """Walrus codegen on this toolchain accepts at most one sync-wait command
    per hardware instruction (plus its update); Tile freely emits several.
    Split the excess waits onto preceding same-engine NoOps."""
    import bass_rust
    import concourse.mybir as mybir

    for f in nc.m.functions:
        for blk in f.blocks:
            insts = blk.instructions
            out = []
            changed = False
            for inst in insts:
                si = inst.sync_info
                if si is not None and si.on_wait and len(si.on_wait) > max_waits:
                    waits = list(si.on_wait)
                    for j, w in enumerate(waits[max_waits:]):
                        nop = mybir.InstNoOp(
                            name=f"{inst.name}-xw{j}",
                            sync_info=bass_rust.SyncInfo(
                                on_wait=[w], on_update=[]
                            ),
                            bass_nofuse=True,
                            engine=inst.engine,
                        )
                        nc.register_instruction(nop, overwrite=True)
                        out.append(nop)
                    inst.sync_info = bass_rust.SyncInfo(
                        on_wait=waits[:max_waits],
                        on_update=list(si.on_update or []),
                    )
                    changed = True
                out.append(inst)
            if changed:
                blk.instructions = out


def _build(lam: float, with_mask: bool, with_qk_bias: bool, with_v_bias: bool,
           split_waits: bool = True):
    import concourse.bass as bass
    import concourse.tile as tile
    import concourse.mybir as mybir

    bf16 = mybir.dt.bfloat16
    f32 = mybir.dt.float32
    AF = mybir.ActivationFunctionType
    ALU = mybir.AluOpType

    SQRT_EPS = float(np.sqrt(EPS))

    nc = bass.Bass()
    xT_d = nc.dram_tensor("xT", [D, L], bf16, kind="ExternalInput")
    wq_d = nc.dram_tensor("wq", [D, COLS], bf16, kind="ExternalInput")
    wk_d = nc.dram_tensor("wk", [D, COLS], bf16, kind="ExternalInput")
    wv_d = nc.dram_tensor("wv", [D, COLS], bf16, kind="ExternalInput")
    wo_d = nc.dram_tensor("wo", [COLS, D], bf16, kind="ExternalInput")
    if with_qk_bias:
        bq_d = nc.dram_tensor("bqs", [COLS], f32, kind="ExternalInput")
        bk_d = nc.dram_tensor("bks", [COLS], f32, kind="ExternalInput")
    if with_v_bias:
        bv_d = nc.dram_tensor("bvs", [COLS], f32, kind="ExternalInput")
    if with_mask:
        maskT_d = nc.dram_tensor("maskT", [L, L], bf16, kind="ExternalInput")
    y_d = nc.dram_tensor("y", [L, D], f32, kind="ExternalOutput")

    with tile.TileContext(nc) as tc:
        persist = tc.alloc_tile_pool(name="persist", bufs=1)
        qT = persist.tile([128, 4, L], bf16)
        kT = persist.tile([128, 4, L], bf16)
        v = persist.tile([128, 8, COLS], bf16)
        wo_s = persist.tile([128, 4, D], bf16)
        outT = persist.tile([128, 4, L], bf16)
        ones = persist.tile([128, 128], bf16)
        nc.vector.memset(ones, 1.0)
        if with_mask:
            ident = persist.tile([128, 128], bf16)
            from concourse.masks import make_identity
            make_identity(nc, ident)
            maskT_s = persist.tile([128, 8, L], bf16)
            nc.sync.dma_start(
                maskT_s, maskT_d.rearrange("(ko p) l -> p ko l", p=128)
            )

        with (
            tc.tile_pool(name="stage_a", bufs=1) as sa,
            tc.tile_pool(name="epool", bufs=2) as ep,
            tc.tile_pool(name="sums", bufs=2) as sp,
            tc.tile_pool(name="t4p", bufs=1) as t4p,
            tc.tile_pool(name="tailp", bufs=2) as tp,
            tc.tile_pool(name="psA", bufs=2, space="PSUM") as psA,
            tc.tile_pool(name="psS", bufs=2, space="PSUM") as psS,
            tc.tile_pool(name="psU", bufs=2, space="PSUM") as psU,
        ):
            xts = sa.tile([128, 8, L], bf16)
            wq_s = sa.tile([128, 8, COLS], bf16)
            wk_s = sa.tile([128, 8, COLS], bf16)
            wv_s = sa.tile([128, 8, COLS], bf16)
            xT_r = xT_d.rearrange("(ko p) l -> p ko l", p=128)
            wq_r = wq_d.rearrange("(ko p) m -> p ko m", p=128)
            wk_r = wk_d.rearrange("(ko p) m -> p ko m", p=128)
            wv_r = wv_d.rearrange("(ko p) m -> p ko m", p=128)
            # x split across sync+gpsimd queues; g0 slices of wq/wk first on
            # scalar/vector queues so the first matmuls' operands land early.
            # x spread over all three DMA-capable queues, g0 weight slices
            # first so the opening qkT units are never transfer-starved
            nc.scalar.dma_start(wq_s[:, :, 0:128], wq_r[:, :, 0:128])
            for kb in range(3):
                nc.sync.dma_start(xts[:, kb], xT_r[:, kb])
            for kb in range(3, 6):
                nc.gpsimd.dma_start(xts[:, kb], xT_r[:, kb])
            nc.scalar.dma_start(wk_s[:, :, 0:128], wk_r[:, :, 0:128])
            nc.scalar.dma_start(xts[:, 6], xT_r[:, 6])
            nc.scalar.dma_start(xts[:, 7], xT_r[:, 7])
            nc.sync.dma_start(wq_s[:, :, 128:512], wq_r[:, :, 128:512])
            nc.gpsimd.dma_start(wk_s[:, :, 128:512], wk_r[:, :, 128:512])
            nc.gpsimd.dma_start(wv_s[:], wv_r[:])
            nc.sync.dma_start(wo_s, wo_d.rearrange("(ko p) n -> p ko n", p=128))
            if with_qk_bias:
                bq_s = sa.tile([128, 4], f32)
                bk_s = sa.tile([128, 4], f32)
                nc.sync.dma_start(bq_s, bq_d.rearrange("(mb p) -> p mb", p=128))
                nc.sync.dma_start(bk_s, bk_d.rearrange("(mb p) -> p mb", p=128))
            if with_v_bias:
                bv_s = sa.tile([128, COLS], f32)
                bv_ap = bv_d[:]
                nc.gpsimd.dma_start(
                    bv_s,
                    bass.AP(
                        tensor=bv_ap.tensor,
                        offset=bv_ap.offset,
                        ap=[[0, 128], list(bv_ap.ap[0])],
                    ),
                )

            # ------------- emission building blocks -------------

            def evac_qk(which, g, lc, acc):
                dst = qT if which == "q" else kT
                dst_ap = dst[:, g, lc * 512:(lc + 1) * 512]
                if with_qk_bias:
                    b_s = bq_s if which == "q" else bk_s
                    nc.scalar.activation(
                        out=dst_ap, in_=acc[:], func=AF.Identity,
                        bias=b_s[:, g:g + 1], scale=1.0,
                    )
                else:
                    nc.vector.tensor_copy(dst_ap, acc[:])

            def unit_qk(g, which, lc):
                # one [128 ch, 512 tok] slice of qT/kT for group g
                wt_s = wq_s if which == "q" else wk_s
                acc = psA.tile([128, 512], f32, tag="acc")
                for kb in range(8):
                    nc.tensor.matmul(
                        acc[:],
                        wt_s[:, kb, g * 128:(g + 1) * 128],
                        xts[:, kb, lc * 512:(lc + 1) * 512],
                        start=(kb == 0),
                        stop=(kb == 7),
                    )
                evac_qk(which, g, lc, acc)

            def emit_qk0_head(g):
                # opening qkT in k-chunk-major order across four parallel
                # accumulators (borrowing the idle psU slots) so each
                # arriving x chunk feeds four matmuls immediately — hides
                # the x transfer tail behind compute
                slots = [("q", 0, psA), ("k", 0, psA), ("q", 1, psU),
                         ("k", 1, psU)]
                accs = []
                for which, lc, pool in slots:
                    accs.append(pool.tile(
                        [128, 512], f32,
                        tag="acc" if pool is psA else "u",
                        name=f"qk0_{which}_{lc}"))
                for kb in range(8):
                    for (which, lc, pool), acc in zip(slots, accs):
                        wt_s = wq_s if which == "q" else wk_s
                        nc.tensor.matmul(
                            acc[:],
                            wt_s[:, kb, g * 128:(g + 1) * 128],
                            xts[:, kb, lc * 512:(lc + 1) * 512],
                            start=(kb == 0),
                            stop=(kb == 7),
                        )
                for (which, lc, pool), acc in zip(slots, accs):
                    evac_qk(which, g, lc, acc)

            def unit_v(lb):
                # v rows [128 tok chunk lb, 512 ch]
                acc = psA.tile([128, 512], f32, tag="acc")
                for kb in range(8):
                    nc.tensor.matmul(
                        acc[:],
                        xts[:, kb, lb * 128:(lb + 1) * 128],
                        wv_s[:, kb, :],
                        start=(kb == 0),
                        stop=(kb == 7),
                    )
                if with_v_bias:
                    nc.vector.tensor_add(v[:, lb, :], acc[:], bv_s[:])
                else:
                    nc.vector.tensor_copy(v[:, lb, :], acc[:])

            def emit_score_mm(g, s, kb, lc, sc):
                nc.tensor.matmul(
                    sc[:, lc * 512:(lc + 1) * 512],
                    kT[64 * s:64 * (s + 1), g, kb * 128:(kb + 1) * 128],
                    qT[64 * s:64 * (s + 1), g, lc * 512:(lc + 1) * 512],
                    start=True,
                    stop=not with_mask,
                    tile_position=(64 * s, 0),
                )
                if with_mask:
                    nc.tensor.matmul(
                        sc[:, lc * 512:(lc + 1) * 512],
                        ident[:],
                        maskT_s[:, kb, lc * 512:(lc + 1) * 512],
                        start=False,
                        stop=True,
                    )

            def emit_scores_kb(g, kb, es_full):
                # all four K=64 score matmuls of this k-chunk land in ONE
                # 4-bank PSUM tile: only the quartet's first matmul carries
                # the tile's drain wait, so the following row-group-disjoint
                # partners issue wait-free and the PE packs each pair
                # concurrently (observed: a waiting matmul never co-issues).
                sc = psS.tile([128, 2 * L], f32, tag="sc", bufs=1,
                              name=f"sc_{g}_{kb}")
                for lc in range(2):
                    for s in range(2):
                        emit_score_mm(g, s, kb, lc, sc[:, s * L:(s + 1) * L])
                # ONE exp drains the whole 4-bank tile: the next chunk's
                # quartet then waits on a single older event, and the 2048
                # free-dim amortizes the ACT fixed overhead
                nc.scalar.activation(
                    out=es_full[:, kb, :], in_=sc[:], func=AF.Exp,
                )

            def emit_lvl1(g, es, t4s, s, j):
                # first-level pair add of the row-sum tree (bf16, 2x DVE)
                nc.vector.tensor_add(
                    t4s[s][:, j], es[s][:, 2 * j], es[s][:, 2 * j + 1]
                )

            def emit_sums(g, t4s):
                # finish tree in-place, then ones-matmul partition reduction
                # leaves row sums replicated across partitions. The copies
                # fold in a sqrt(eps) scale: S' = sqrt(eps)*S makes the RMS
                # eps-term simply (S0'*S1')^2 (the S scale itself cancels
                # between z and the denominator), and bf16 S is safe for the
                # same cancellation reason.
                srep_ps = psS.tile([128, 2 * L], f32, tag="sc", bufs=1,
                                   name=f"srep_ps_{g}")
                reps = []
                for s in range(2):
                    t4 = t4s[s]
                    nc.vector.tensor_add(t4[:, 0], t4[:, 0], t4[:, 1])
                    nc.vector.tensor_add(t4[:, 2], t4[:, 2], t4[:, 3])
                    nc.vector.tensor_add(t4[:, 0], t4[:, 0], t4[:, 2])
                    for lc in range(2):
                        nc.tensor.matmul(
                            srep_ps[:, s * L + lc * 512:s * L + (lc + 1) * 512],
                            ones[:],
                            t4[:, 0, lc * 512:(lc + 1) * 512],
                            start=True,
                            stop=True,
                        )
                    srep = sp.tile([128, L], bf16, tag=f"s{s}",
                                   name=f"srep_{g}_{s}")
                    with nc.allow_low_precision(reason="S scale cancels"):
                        nc.scalar.activation(
                            out=srep[:], in_=srep_ps[:, s * L:(s + 1) * L],
                            func=AF.Identity, scale=SQRT_EPS,
                        )
                    reps.append(srep)
                return reps

            def unit_u_half(g, lc, s, es):
                # u_s = v^T e_s for one 512-token q chunk, one s stream
                cs = slice(lc * 512, (lc + 1) * 512)
                pool = psA if (g == 3 and lc == 1) else psU
                u = pool.tile([128, 512], f32,
                              tag="acc" if pool is psA else "u",
                              name=f"u_{g}_{lc}_{s}")
                for kb in range(8):
                    nc.tensor.matmul(
                        u[:],
                        v[:, kb, 128 * g:128 * (g + 1)],
                        es[s][:, kb, cs],
                        start=(kb == 0),
                        stop=(kb == 7),
                    )
                return u

            def emit_combine(g, lc, us, reps, z):
                cs = slice(lc * 512, (lc + 1) * 512)
                s0_rep, s1_rep = reps
                tt = tp.tile([128, 512], bf16, tag="tt", name=f"tt_{g}_{lc}")
                with nc.allow_low_precision(reason="bf16 z, scale cancels"):
                    nc.vector.tensor_mul(z[:, cs], us[0][:], s1_rep[:, cs])
                    nc.vector.scalar_tensor_tensor(
                        out=tt[:], in0=us[1][:], scalar=lam,
                        in1=s0_rep[:, cs], op0=ALU.mult, op1=ALU.mult,
                    )
                    nc.vector.tensor_sub(z[:, cs], z[:, cs], tt[:])

            def emit_tail_ctwt(g, reps):
                # (S0'*S1')^2 = eps*(S0*S1)^2 on replicated tiles (the
                # sqrt(eps) scale was folded into the S copies); depends
                # only on sums, so it can be emitted before the combines.
                s0_rep, s1_rep = reps
                ct = tp.tile([128, L], bf16, tag="ct", name=f"ct_{g}")
                wt = tp.tile([128, L], bf16, tag="wt", name=f"wt_{g}")
                with nc.allow_low_precision(reason="eps term only"):
                    nc.vector.tensor_mul(ct[:], s1_rep[:], s0_rep[:])
                    nc.vector.tensor_mul(wt[:], ct[:], ct[:])
                return ct, wt

            def emit_tail(g, z, reps, ctwt=None, sq_pool="u"):
                # headwise RMS over the 128-partition channel dim, all on
                # replicated [128, L] tiles: out = z * rsqrt(sum(z^2)/128
                # + eps*(S0*S1)^2), with rsqrt = exp(-0.5*ln(.)) so ScalarE
                # stays on the natural_log_exp table set. The z^2 partition
                # sums go through psU tiles so the score-PSUM banks drain
                # early for the output projection's early start.
                if ctwt is None:
                    ctwt = emit_tail_ctwt(g, reps)
                ct, wt = ctwt
                zsq = tp.tile([128, L], bf16, tag="zr", name=f"zsq_{g}")
                lnw = tp.tile([128, L], f32, tag="lnw", name=f"lnw_{g}")
                rsq = tp.tile([128, L], bf16, tag="zr", name=f"rsq_{g}")
                for lc in range(2):
                    cs = slice(lc * 512, (lc + 1) * 512)
                    pool = psA if sq_pool == "acc" else psU
                    sq_ps = pool.tile([128, 512], f32, tag=sq_pool,
                                      name=f"sq_ps_{g}_{lc}")
                    with nc.allow_low_precision(reason="bf16 z^2 RMS sum"):
                        nc.vector.tensor_mul(zsq[:, cs], z[:, cs], z[:, cs])
                    nc.tensor.matmul(
                        sq_ps[:], ones[:], zsq[:, cs],
                        start=True, stop=True,
                    )
                    # lnw = ln(sq/128 + eps*(S0*S1)^2)  (f32: rsq is exp of
                    # -lnw/2, so lnw needs absolute accuracy ~1e-3)
                    nc.vector.scalar_tensor_tensor(
                        out=lnw[:, cs], in0=sq_ps[:], scalar=1.0 / 128,
                        in1=wt[:, cs], op0=ALU.mult, op1=ALU.add,
                    )
                    nc.scalar.activation(
                        out=lnw[:, cs], in_=lnw[:, cs], func=AF.Ln,
                    )
                    with nc.allow_low_precision(reason="bf16 rsqrt bcast"):
                        nc.scalar.activation(
                            out=rsq[:, cs], in_=lnw[:, cs], func=AF.Exp,
                            scale=-0.5,
                        )
                        nc.vector.tensor_mul(
                            outT[:, g, cs], z[:, cs], rsq[:, cs]
                        )

            # ------------- interleaved emission schedule -------------
            # scores/exp for group g pace each loop; qkT/v/u units slot in
            # between the kb steps so the PE stream stays dense.

            emit_qk0_head(0)

            es_all = {}
            t4_all = {}
            sums_all = {}
            z_all = {}
            u_tiles = {}

            es_full_all = {}

            def new_g_state(g):
                es_full = ep.tile([128, 8, 2 * L], bf16, tag="e",
                                  name=f"e_{g}")
                es_full_all[g] = es_full
                es_all[g] = [es_full[:, :, 0:L], es_full[:, :, L:2 * L]]
                t4_all[g] = [
                    t4p.tile([128, 4, L], bf16, tag="t40", name=f"t40_{g}"),
                    t4p.tile([128, 4, L], bf16, tag="t41", name=f"t41_{g}"),
                ]

            def pull_u(g, lc, s):
                u_tiles[(g, lc, s)] = unit_u_half(g, lc, s, es_all[g])
                if s == 1 and g < 3:
                    emit_combine(
                        g, lc,
                        [u_tiles[(g, lc, 0)], u_tiles[(g, lc, 1)]],
                        sums_all[g], z_all[g],
                    )

            def u3_mms(lc, s, kbs):
                # u(3) incremental accumulation: each matmul is emitted the
                # iteration after its exp exists, keeping g3's late score
                # iterations backfilled (has_written persists in the bank)
                cs = slice(lc * 512, (lc + 1) * 512)
                if (3, lc, s) not in u_tiles:
                    pool = psA if lc == 1 else psU
                    u_tiles[(3, lc, s)] = pool.tile(
                        [128, 512], f32,
                        tag="acc" if pool is psA else "u",
                        name=f"u3p_{lc}_{s}")
                u = u_tiles[(3, lc, s)]
                for kb in kbs:
                    nc.tensor.matmul(
                        u[:], v[:, kb, 384:512], es_all[3][s][:, kb, cs],
                        start=(kb == 0), stop=(kb == 7),
                    )

            # background unit schedule, one ~8-matmul pull per score
            # iteration so the PE stream stays dense past the exp pace
            pulls = {
                0: [lambda: unit_qk(1, "q", 0), lambda: unit_qk(1, "k", 0),
                    lambda: unit_qk(1, "q", 1), lambda: unit_qk(1, "k", 1),
                    lambda: unit_v(0), lambda: unit_v(1),
                    lambda: unit_v(2), lambda: unit_v(3)],
                1: [lambda: unit_v(4), lambda: unit_v(5),
                    lambda: unit_v(6), lambda: unit_v(7),
                    lambda: pull_u(0, 0, 0), lambda: pull_u(0, 0, 1),
                    lambda: pull_u(0, 1, 0), lambda: pull_u(0, 1, 1)],
                2: [lambda: pull_u(1, 0, 0), lambda: pull_u(1, 0, 1),
                    lambda: pull_u(1, 1, 0), lambda: pull_u(1, 1, 1),
                    lambda: (emit_tail(0, z_all[0], sums_all[0]),
                             unit_qk(3, "q", 0)),
                    lambda: (unit_qk(3, "k", 0),
                             emit_tail(1, z_all[1], sums_all[1])),
                    lambda: unit_qk(3, "q", 1), lambda: unit_qk(3, "k", 1)],
                3: [lambda: pull_u(2, 0, 0), lambda: pull_u(2, 0, 1),
                    lambda: pull_u(2, 1, 0), lambda: pull_u(2, 1, 1),
                    lambda: (emit_tail(2, z_all[2], sums_all[2],
                                       sq_pool="acc"),
                             u3_mms(0, 0, range(0, 4))),
                    lambda: (u3_mms(0, 1, range(0, 4)),
                             u3_mms(0, 0, range(4, 5))),
                    lambda: (u3_mms(1, 0, range(0, 5)),
                             u3_mms(0, 1, range(4, 5))),
                    lambda: (u3_mms(1, 1, range(0, 6)),
                             u3_mms(0, 0, range(5, 7)),
                             u3_mms(0, 1, range(5, 6)))],
            }

            for g in range(4):
                new_g_state(g)
                z_all[g] = tp.tile([128, L], bf16, tag="z", name=f"z_{g}")
                es = es_all[g]
                for kb in range(8):
                    emit_scores_kb(g, kb, es_full_all[g])
                    if kb % 2 == 1:
                        emit_lvl1(g, es, t4_all[g], 0, kb // 2)
                        emit_lvl1(g, es, t4_all[g], 1, kb // 2)
                    pull = pulls[g][kb]
                    if pull is not None:
                        pull()
                if g == 3:
                    # finish the incremental u(3) groups (trailing k-chunks)
                    u3_mms(0, 0, (7,))
                    u3_mms(0, 1, (6, 7))
                    u3_mms(1, 0, (5, 6, 7))
                    u3_mms(1, 1, (6, 7))
                sums_all[g] = emit_sums(g, t4_all[g])
                if g == 1:
                    # qkT for group 2 must precede scores(2) on the PE
                    unit_qk(2, "q", 0)
                    unit_qk(2, "k", 0)
                    unit_qk(2, "q", 1)
                    unit_qk(2, "k", 1)

            # ---------------- drain + output projection ----------------
            # Stage D y-tiles borrow the score-tag [128, 2048] PSUM tiles
            # (4 y accumulators per alloc) inside the main block, so there
            # is no pool barrier: the g=0..2 accumulations stream during
            # the combine/tail DVE chain and the g=3 row lands per tile as
            # soon as outT[:, 3] is written.
            y_r = y_d.rearrange("(lb p) n -> p lb n", p=128)
            tiles = [(lb, nk) for lb in range(8) for nk in range(2)]

            # drain group 3: eps term first (needs only sums), then the two
            # combine halves feeding the tail pipeline
            ctwt3 = emit_tail_ctwt(3, sums_all[3])
            emit_combine(3, 0, [u_tiles[(3, 0, 0)], u_tiles[(3, 0, 1)]],
                         sums_all[3], z_all[3])
            emit_combine(3, 1, [u_tiles[(3, 1, 0)], u_tiles[(3, 1, 1)]],
                         sums_all[3], z_all[3])
            emit_tail(3, z_all[3], sums_all[3], ctwt=ctwt3)

        # ---------------- Stage D: output projection ----------------
        # first tiles accumulate g=0..2 while tail(3) computes; the g=3
        # contribution and evacuation follow once outT[:, 3] lands.
        with (
            tc.tile_pool(name="yp", bufs=4) as yp,
            tc.tile_pool(name="psY", bufs=8, space="PSUM") as psY,
        ):
            y_r = y_d.rearrange("(lb p) n -> p lb n", p=128)
            tiles = [(lb, nk) for lb in range(8) for nk in range(2)]
            accs = {}
            for i, (lb, nk) in enumerate(tiles[:8]):
                acc = psY.tile([128, 512], f32, tag="y")
                accs[(lb, nk)] = acc
                for gg in range(3):
                    nc.tensor.matmul(
                        acc[:],
                        outT[:, gg, lb * 128:(lb + 1) * 128],
                        wo_s[:, gg, nk * 512:(nk + 1) * 512],
                        start=(gg == 0),
                        stop=False,
                    )
            for i, (lb, nk) in enumerate(tiles):
                if (lb, nk) in accs:
                    acc = accs[(lb, nk)]
                    nc.tensor.matmul(
                        acc[:],
                        outT[:, 3, lb * 128:(lb + 1) * 128],
                        wo_s[:, 3, nk * 512:(nk + 1) * 512],
                        start=False,
                        stop=True,
                    )
                else:
                    acc = psY.tile([128, 512], f32, tag="y")
                    for gg in range(4):
                        nc.tensor.matmul(
                            acc[:],
                            outT[:, gg, lb * 128:(lb + 1) * 128],
                            wo_s[:, gg, nk * 512:(nk + 1) * 512],
                            start=(gg == 0),
                            stop=(gg == 3),
                        )
                yt = yp.tile([128, 512], f32, tag="yt")
                if i % 2 == 0:
                    nc.scalar.copy(out=yt[:], in_=acc[:])
                else:
                    nc.vector.tensor_copy(yt[:], acc[:])
                q = nc.sync if i % 2 == 0 else nc.gpsimd
                q.dma_start(y_r[:, lb, nk * 512:(nk + 1) * 512], yt[:])

        persist.release()
    if split_waits:
        _split_excess_waits(nc)
    return nc


def kernel(**inputs) -> np.ndarray:
    from concourse.bass_utils import run_bass_kernel_spmd

    bf = ml_dtypes.bfloat16
    q_in = np.asarray(inputs["query"], np.float32)      # (L, NB, D)
    Wq = np.asarray(inputs["Wq"], np.float32)
    Wk = np.asarray(inputs["Wk"], np.float32)
    Wv = np.asarray(inputs["Wv"], np.float32)
    Wo = np.asarray(inputs["Wo"], np.float32)
    bq = np.asarray(inputs["bq"], np.float32)
    bk = np.asarray(inputs["bk"], np.float32)
    bv = np.asarray(inputs["bv"], np.float32)
    bo = np.asarray(inputs["bo"], np.float32)
    norm_w = np.asarray(inputs["norm_w"], np.float32)
    mask = np.asarray(inputs["attn_mask"], np.float32)
    lq1 = np.asarray(inputs["lq1"], np.float32)
    lk1 = np.asarray(inputs["lk1"], np.float32)
    lq2 = np.asarray(inputs["lq2"], np.float32)
    lk2 = np.asarray(inputs["lk2"], np.float32)

    lam = float(
        np.exp(np.sum(lq1 * lk1)) - np.exp(np.sum(lq2 * lk2)) + LAMBDA_INIT
    )
    scale = HD ** -0.5
    with_mask = bool(np.any(mask))
    with_qk_bias = bool(np.any(bq) or np.any(bk))
    with_v_bias = bool(np.any(bv))
    # norm_w * (1 - lambda_init) folded into Wo rows (tiled per he-head)
    nw = np.tile(norm_w * (1.0 - LAMBDA_INIT), HE // 2)  # (COLS,)

    nc = _build(lam, with_mask, with_qk_bias, with_v_bias)

    maskT = np.ascontiguousarray(mask.T).astype(bf) if with_mask else None
    in_maps = []
    for c in range(NCORES):
        b, g2 = divmod(c, 2)
        cols = slice(COLS * g2, COLS * (g2 + 1))
        x = q_in[:, b, :]
        im = {
            "xT": np.ascontiguousarray(x.T).astype(bf),
            "wq": (Wq[:, cols] * scale).astype(bf),
            "wk": np.ascontiguousarray(Wk[:, cols]).astype(bf),
            "wv": np.ascontiguousarray(Wv[:, cols]).astype(bf),
            "wo": (Wo[cols, :] * nw[:, None]).astype(bf),
        }
        if with_qk_bias:
            im["bqs"] = np.ascontiguousarray(bq[cols] * scale)
            im["bks"] = np.ascontiguousarray(bk[cols])
        if with_v_bias:
            im["bvs"] = np.ascontiguousarray(bv[cols])
        if with_mask:
            im["maskT"] = maskT
        in_maps.append(im)

    res = run_bass_kernel_spmd(nc, in_maps, core_ids=list(range(NCORES)))
    global LAST_RESULT
    LAST_RESULT = res
    outs = [r["y"] for r in res.results]

    out = np.empty((L, NB, D), np.float32)
    for b in range(NB):
        yb = outs[2 * b] + outs[2 * b + 1]
        if np.any(bo):
            yb = yb + bo
        out[:, b, :] = yb
    return out
